# revision 28
# baseline (speedup 1.0000x reference)
"""Multi-head attention + output projection on 8 Trainium2 NeuronCores.

Problem (hardcoded): x [3, 2, 4096, 512] fp32 (q/k/v stacked), proj_w [512, 512],
proj_b [512].  reference = softmax(q k^T / sqrt(64)) v, heads=8, then
out @ proj_w.T + proj_b.

Sharding: B*H = 16 (batch, head) pairs over 8 cores -> each core gets one
batch and one adjacent head PAIR (2 heads = 128 feature dims).  The output
projection is tensor-parallel along the contraction dim: each core computes
its partial y = attn_out_pair @ W[:, pair_dims].T; the host sums the 4
partials per batch and adds the bias.

Device kernel (per core).  Exp of 33.5M scores/core is the dominant cost
(ScalarE streams 1 elem/cycle/lane + ~350 cyc per ACT), so the softmax is
split across two engines and the PE stream is organized to amortize its
row-config switch penalty:
  - scores (fp16 q/k, row-tiled head pairs) land in a 6-bank PSUM ring of
    three [128, 1024] chunk slots; emission batches 3 chunks of scores,
    then 6 PV matmuls, so the K=64 <-> K=128 array-reconfig drain (~165ns)
    is paid twice per 3 chunks instead of twice per chunk.
  - exp slots cycle through EXP_PATTERN: 'S' = exact ScalarE ACT (fp16
    out), 'D' = DVE Schraudolph bit-trick exp -- one tensor_scalar
    computing int16(A*s + B) whose bits are read back as bf16, i.e.
    2^(x/log2) by exponent-field construction (~1.8% rms, metric impact
    ~1e-2 at 1/3 of keys; the denominator uses the same approximate P, so
    softmax errors partially cancel as a reweighting).
  - PV accumulates [V | 1] fp16/bf16 (denominator rides as the 65th
    column) into two PSUM banks; the projection time-shares those banks
    between blocks (all projs of block b are emitted before PV of block
    b+1 re-arms the accumulation group).
"""

import numpy as np

C, B, N, D, H = 3, 2, 4096, 512, 8
HD = 64          # head dim
NCORES = 8
NQB = 512        # nq block width
NBLK = N // NQB  # 8 q blocks
NCHUNK = N // 128  # 32 nk chunks of 128
NSLOTS = NBLK * NCHUNK             # 256 slots; 1 slot = 1 chunk = [128, 1024]

# exp engine pattern, cycled per slot: 'S' = ScalarE exact, 'D' = DVE
# Schraudolph approximation.  'SSD' puts 1/3 of exp on the DVE.
EXP_PATTERN = "SSD"

# Schraudolph constants: exp(0.125*s) ~= bitcast_bf16(int16(SCH_A*s + SCH_B))
SCH_A = float(0.125 * (1 << 7) / np.log(2))
SCH_B = float(127.0 * (1 << 7) - 7.42)

_compiled = None


def _build_nc():
    import concourse.bacc as bacc
    import concourse.tile as tile
    from concourse import mybir

    f32 = mybir.dt.float32
    f32r = mybir.dt.float32r
    fp16 = mybir.dt.float16
    bf16 = mybir.dt.bfloat16
    i16 = mybir.dt.int16
    Exp = mybir.ActivationFunctionType.Exp
    mult = mybir.AluOpType.mult
    add = mybir.AluOpType.add

    nc = bacc.Bacc("TRN2", target_bir_lowering=False, debug=False, num_devices=1)

    qT = nc.dram_tensor("qT", [128, N], fp16, kind="ExternalInput").ap()
    kT = nc.dram_tensor("kT", [128, N], fp16, kind="ExternalInput").ap()
    vIh = nc.dram_tensor("vIh", [128, NCHUNK, 2, HD + 1], fp16, kind="ExternalInput").ap()
    vIb = nc.dram_tensor("vIb", [128, NCHUNK, 2, HD + 1], bf16, kind="ExternalInput").ap()
    wT = nc.dram_tensor("wT", [HD, 2, D], fp16, kind="ExternalInput").ap()
    y = nc.dram_tensor("y", [N, D], f32, kind="ExternalOutput").ap()

    # slot -> exp engine
    def slot_engine(j):
        return EXP_PATTERN[j % len(EXP_PATTERN)]

    with tile.TileContext(nc) as tc:
        with (
            tc.tile_pool(name="const", bufs=1) as const_pool,
            tc.tile_pool(name="pt", bufs=10) as pt_pool,
            tc.tile_pool(name="ptd", bufs=5) as ptd_pool,
            tc.tile_pool(name="ep", bufs=2) as ep_pool,
            tc.tile_pool(name="ps_s", bufs=1, space="PSUM") as ps_s,
            tc.tile_pool(name="ps_a", bufs=1, space="PSUM") as ps_a,
            tc.tile_pool(name="dram", bufs=2, space="DRAM") as dram_pool,
        ):
            # ---- resident inputs ----
            qT_sb = const_pool.tile([128, N], fp16)
            kT_sb = const_pool.tile([128, N], fp16)
            vIh_sb = const_pool.tile([128, NCHUNK, 2, HD + 1], fp16)
            vIb_sb = const_pool.tile([128, NCHUNK, 2, HD + 1], bf16)
            # trigger the exp table load while the input DMAs stream in
            warm = ep_pool.tile([128, 2], f32, tag="warm")
            nc.vector.memset(warm[:], 0.0)
            nc.scalar.activation(warm[:, 1:2], warm[:, 0:1], Exp)
            nc.sync.dma_start(kT_sb[:, 0:128], kT[:, 0:128])
            nc.sync.dma_start(qT_sb[:, 0:NQB], qT[:, 0:NQB])
            nc.sync.dma_start(kT_sb[:, 128:512], kT[:, 128:512])
            nc.gpsimd.dma_start(vIh_sb[:, 0:4], vIh[:, 0:4])
            nc.gpsimd.dma_start(vIb_sb[:, 0:4], vIb[:, 0:4])
            for ck4 in range(4, NCHUNK, 4):
                sl = slice(ck4 * 128, (ck4 + 4) * 128)
                nc.sync.dma_start(kT_sb[:, sl], kT[:, sl])
                nc.gpsimd.dma_start(vIh_sb[:, ck4:ck4 + 4], vIh[:, ck4:ck4 + 4])
                nc.gpsimd.dma_start(vIb_sb[:, ck4:ck4 + 4], vIb[:, ck4:ck4 + 4])
            for b in range(1, NBLK):
                nc.gpsimd.dma_start(qT_sb[:, b * NQB:(b + 1) * NQB],
                                    qT[:, b * NQB:(b + 1) * NQB])
            wT_sb = const_pool.tile([HD, 2, D], fp16)
            nc.sync.dma_start(wT_sb[:], wT[:])
            zeros_sb = const_pool.tile([128, NQB], mybir.dt.bfloat16)
            nc.vector.memset(zeros_sb[:], 0.0)

            # ---- PSUM ----
            # score ring: 3 slots x [128, 1024] fp32 (2 banks each)
            ring = [ps_s.tile([128, 1024], f32, tag=f"ring{i}", name=f"ring{i}")
                    for i in range(3)]
            # accumulators / proj outputs: 2 banks, time-shared.
            acc = [ps_a.tile([128, NQB], f32, tag=f"acc{i}", name=f"acc{i}")
                   for i in range(2)]

            # HAM warm: dense burst of zero matmuls (wiped by start=True PV)
            for f in range(20):
                nc.tensor.matmul(acc[f % 2][0:HD + 1, :],
                                 lhsT=zeros_sb[:, 0:HD + 1], rhs=zeros_sb[:],
                                 start=False, stop=False)

            # ---- helpers ----
            def emit_scores(j):
                """scores matmul pair for slot j = chunk (blk, ck)."""
                blk, ck = divmod(j, NCHUNK)
                for h in range(2):
                    nc.tensor.matmul(
                        ring[j % 3][:, h * 512:(h + 1) * 512],
                        lhsT=kT_sb[h * HD:(h + 1) * HD, ck * 128:(ck + 1) * 128],
                        rhs=qT_sb[h * HD:(h + 1) * HD, blk * NQB:(blk + 1) * NQB],
                        start=True, stop=True)

            pt_of_slot = {}

            def emit_exp(j):
                src = ring[j % 3][:]
                if slot_engine(j) == "S":
                    pt = pt_pool.tile([128, 1024], fp16, tag="ptS")
                    nc.scalar.activation(pt[:], src, Exp, scale=0.125)
                else:
                    pt = ptd_pool.tile([128, 1024], i16, tag="ptD")
                    nc.vector.tensor_scalar(
                        pt[:], src, SCH_A, SCH_B, op0=mult, op1=add)
                pt_of_slot[j] = pt

            def emit_pv(blk, ck):
                j = blk * NCHUNK + ck
                pt = pt_of_slot.pop(j)
                for h in range(2):
                    sl = pt[:, h * 512:(h + 1) * 512]
                    if slot_engine(j) == "S":
                        rhs, lhsT = sl, vIh_sb[:, ck, h, :]
                    else:
                        rhs, lhsT = sl.bitcast(bf16), vIb_sb[:, ck, h, :]
                    nc.tensor.matmul(
                        acc[h][0:HD + 1, :], lhsT=lhsT, rhs=rhs,
                        start=(ck == 0), stop=(ck == NCHUNK - 1))

            def emit_stage(blk):
                """stage accumulators to SBUF + denominator transpose/recip."""
                st0 = ep_pool.tile([HD + 1, NQB], fp16, tag="st0")
                nc.vector.tensor_copy(st0[:], acc[0][0:HD + 1, :])
                dtmp = dram_pool.tile([2, NQB], fp16, tag="dtmp")
                nc.sync.dma_start(dtmp[0:1, :], st0[HD:HD + 1, :])
                st1 = ep_pool.tile([HD + 1, NQB], fp16, tag="st1")
                nc.vector.tensor_copy(st1[:], acc[1][0:HD + 1, :])
                nc.sync.dma_start(dtmp[1:2, :], st1[HD:HD + 1, :])
                dT = ep_pool.tile([128, 4, 2], fp16, tag="dT")
                for h in range(2):
                    nc.sync.dma_start(
                        dT[:, :, h], dtmp[h].rearrange("(c p) -> p c", p=128))
                denT = ep_pool.tile([128, 4, 2], f32, tag="denT")
                nc.vector.reciprocal(denT[:], dT[:])
                return st0, st1, denT

            def make_proj(cc, st0, st1, denT, blk):
                def emit_proj():
                    n0 = blk * NQB + cc * 128
                    y0 = acc[0][:, :]
                    nc.tensor.matmul(
                        y0, lhsT=st0[0:HD, cc * 128:(cc + 1) * 128],
                        rhs=wT_sb[:, 0, :], start=True, stop=True)
                    y1 = acc[1][:, :]
                    nc.tensor.matmul(
                        y1, lhsT=st1[0:HD, cc * 128:(cc + 1) * 128],
                        rhs=wT_sb[:, 1, :], start=True, stop=True)
                    t1 = ep_pool.tile([128, D], f32, tag="t1")
                    nc.scalar.mul(t1[:], y1, denT[:, cc, 1:2])
                    y_sb = ep_pool.tile([128, D], f32, tag="y_sb")
                    nc.vector.scalar_tensor_tensor(
                        y_sb[:], y0, denT[:, cc, 0:1], t1[:], op0=mult, op1=add)
                    nc.sync.dma_start(y[n0:n0 + 128, :], y_sb[:])
                return emit_proj

            # ---- main slot loop ----
            from collections import deque
            pv_ready = deque()      # (blk, ck) with both exps emitted
            pending_projs = deque()
            done_units = 0

            def drain(j):
                """emit PV work whose exp slot index < j, plus due projs."""
                nonlocal done_units
                while pending_projs and pending_projs[0][0] <= j:
                    pending_projs.popleft()[1]()
                while pv_ready and pv_ready[0][2] < j:
                    blk, ck, _ = pv_ready.popleft()
                    if ck == 0:
                        # acc banks are about to be re-armed (start=True):
                        # every proj reading them must be emitted first
                        while pending_projs:
                            pending_projs.popleft()[1]()
                    emit_pv(blk, ck)
                    done_units += 2
                    if ck == NCHUNK - 1:
                        st0, st1, denT = emit_stage(blk)
                        for cc in range(4):
                            pending_projs.append(
                                (j + 1 + cc, make_proj(cc, st0, st1, denT, blk)))

            for g in range(0, NSLOTS, 3):
                for j in range(g, min(g + 3, NSLOTS)):
                    emit_scores(j)
                for j in range(g, min(g + 3, NSLOTS)):
                    emit_exp(j)
                    blk, ck = divmod(j, NCHUNK)
                    pv_ready.append((blk, ck, j))
                drain(g)
            drain(NSLOTS + 3)
            while pending_projs:
                pending_projs.popleft()[1]()

    nc.compile()
    return nc


def _get_compiled():
    global _compiled
    if _compiled is None:
        _compiled = _build_nc()
    return _compiled


def _prep_core_inputs(x, proj_w):
    """Host-side shard + layout per core: core c -> batch c//4, head pair c%4."""
    ins = []
    for c in range(NCORES):
        b, hp = c // 4, c % 4
        sl = slice(128 * hp, 128 * hp + 128)
        qTa = np.ascontiguousarray(x[0, b, :, sl].T).astype(np.float16)
        kTa = np.ascontiguousarray(x[1, b, :, sl].T).astype(np.float16)
        v = x[2, b, :, sl]                       # [N, 128]
        vI = np.ones((128, NCHUNK, 2, HD + 1), np.float32)
        vr = v.reshape(NCHUNK, 128, 2, HD)        # [chunk, p, head, m]
        vI[:, :, :, :HD] = vr.transpose(1, 0, 2, 3)
        wTa = np.ascontiguousarray(
            proj_w[:, sl].T.reshape(2, HD, D).transpose(1, 0, 2)).astype(np.float16)
        import ml_dtypes
        ins.append({"qT": qTa, "kT": kTa, "vIh": vI.astype(np.float16),
                    "vIb": vI.astype(ml_dtypes.bfloat16), "wT": wTa})
    return ins


def kernel(x, proj_w, proj_b):
    from concourse.bass_utils import run_bass_kernel_spmd

    x = np.asarray(x, dtype=np.float32)
    proj_w = np.asarray(proj_w, dtype=np.float32)
    proj_b = np.asarray(proj_b, dtype=np.float32)

    nc = _get_compiled()
    in_maps = _prep_core_inputs(x, proj_w)
    res = run_bass_kernel_spmd(nc, in_maps, core_ids=list(range(NCORES)))

    out = np.zeros((B, N, D), np.float32)
    for c in range(NCORES):
        out[c // 4] += res.results[c]["y"]
    out += proj_b
    return out


# revision 30
# speedup vs baseline: 1.0073x; 1.0073x over previous
"""Multi-head attention + output projection on 8 Trainium2 NeuronCores.

Problem (hardcoded): x [3, 2, 4096, 512] fp32 (q/k/v stacked), proj_w [512, 512],
proj_b [512].  reference = softmax(q k^T / sqrt(64)) v, heads=8, then
out @ proj_w.T + proj_b.

Sharding: B*H = 16 (batch, head) pairs over 8 cores -> each core gets one
batch and one adjacent head PAIR (2 heads = 128 feature dims).  The output
projection is tensor-parallel along the contraction dim: each core computes
its partial y = attn_out_pair @ W[:, pair_dims].T; the host sums the 4
partials per batch and adds the bias.

Device kernel (per core).  Exp of 33.5M scores/core is the dominant cost
(ScalarE streams 1 elem/cycle/lane + ~350 cyc per ACT), so the softmax is
split across two engines and the PE stream is organized to amortize its
row-config switch penalty:
  - scores (fp16 q/k, row-tiled head pairs) land in a 6-bank PSUM ring of
    three [128, 1024] chunk slots; emission batches 3 chunks of scores,
    then 6 PV matmuls, so the K=64 <-> K=128 array-reconfig drain (~165ns)
    is paid twice per 3 chunks instead of twice per chunk.
  - exp slots cycle through EXP_PATTERN: 'S' = exact ScalarE ACT (fp16
    out), 'D' = DVE Schraudolph bit-trick exp -- one tensor_scalar
    computing int16(A*s + B) whose bits are read back as bf16, i.e.
    2^(x/log2) by exponent-field construction (~1.8% rms, metric impact
    ~1e-2 at 1/3 of keys; the denominator uses the same approximate P, so
    softmax errors partially cancel as a reweighting).
  - PV accumulates [V | 1] fp16/bf16 (denominator rides as the 65th
    column) into two PSUM banks; the projection time-shares those banks
    between blocks (all projs of block b are emitted before PV of block
    b+1 re-arms the accumulation group).
"""

import numpy as np

C, B, N, D, H = 3, 2, 4096, 512, 8
HD = 64          # head dim
NCORES = 8
NQB = 512        # nq block width
NBLK = N // NQB  # 8 q blocks
NCHUNK = N // 128  # 32 nk chunks of 128
NSLOTS = NBLK * NCHUNK             # 256 slots; 1 slot = 1 chunk = [128, 1024]

# exp engine pattern, cycled per slot: 'S' = ScalarE exact, 'D' = DVE
# Schraudolph approximation.  'SSD' puts 1/3 of exp on the DVE.
EXP_PATTERN = "SSD"

# Schraudolph constants: exp(0.125*s) ~= bitcast_bf16(int16(SCH_A*s + SCH_B))
SCH_A = float(0.125 * (1 << 7) / np.log(2))
SCH_B = float(127.0 * (1 << 7) - 7.42)

_compiled = None


def _build_nc():
    import concourse.bacc as bacc
    import concourse.tile as tile
    from concourse import mybir

    f32 = mybir.dt.float32
    f32r = mybir.dt.float32r
    fp16 = mybir.dt.float16
    bf16 = mybir.dt.bfloat16
    i16 = mybir.dt.int16
    Exp = mybir.ActivationFunctionType.Exp
    mult = mybir.AluOpType.mult
    add = mybir.AluOpType.add

    nc = bacc.Bacc("TRN2", target_bir_lowering=False, debug=False, num_devices=1)

    qT = nc.dram_tensor("qT", [128, N], fp16, kind="ExternalInput").ap()
    kT = nc.dram_tensor("kT", [128, N], fp16, kind="ExternalInput").ap()
    vIh = nc.dram_tensor("vIh", [128, NCHUNK, 2, HD + 1], fp16, kind="ExternalInput").ap()
    vIb = nc.dram_tensor("vIb", [128, NCHUNK, 2, HD + 1], bf16, kind="ExternalInput").ap()
    wT = nc.dram_tensor("wT", [HD, 2, D], fp16, kind="ExternalInput").ap()
    y = nc.dram_tensor("y", [N, D], f32, kind="ExternalOutput").ap()

    # slot -> exp engine
    def slot_engine(j):
        return EXP_PATTERN[j % len(EXP_PATTERN)]

    with tile.TileContext(nc) as tc:
        with (
            tc.tile_pool(name="const", bufs=1) as const_pool,
            tc.tile_pool(name="pt", bufs=10) as pt_pool,
            tc.tile_pool(name="ptd", bufs=5) as ptd_pool,
            tc.tile_pool(name="ep", bufs=2) as ep_pool,
            tc.tile_pool(name="ps_s", bufs=1, space="PSUM") as ps_s,
            tc.tile_pool(name="ps_a", bufs=1, space="PSUM") as ps_a,
            tc.tile_pool(name="dram", bufs=2, space="DRAM") as dram_pool,
        ):
            # ---- resident inputs ----
            qT_sb = const_pool.tile([128, N], fp16)
            kT_sb = const_pool.tile([128, N], fp16)
            vIh_sb = const_pool.tile([128, NCHUNK, 2, HD + 1], fp16)
            vIb_sb = const_pool.tile([128, NCHUNK, 2, HD + 1], bf16)
            # trigger the exp table load while the input DMAs stream in
            warm = ep_pool.tile([128, 2], f32, tag="warm")
            nc.vector.memset(warm[:], 0.0)
            nc.scalar.activation(warm[:, 1:2], warm[:, 0:1], Exp)
            nc.sync.dma_start(kT_sb[:, 0:128], kT[:, 0:128])
            nc.sync.dma_start(qT_sb[:, 0:NQB], qT[:, 0:NQB])
            nc.sync.dma_start(kT_sb[:, 128:512], kT[:, 128:512])
            nc.gpsimd.dma_start(vIh_sb[:, 0:4], vIh[:, 0:4])
            nc.gpsimd.dma_start(vIb_sb[:, 0:4], vIb[:, 0:4])
            for ck4 in range(4, NCHUNK, 4):
                sl = slice(ck4 * 128, (ck4 + 4) * 128)
                nc.sync.dma_start(kT_sb[:, sl], kT[:, sl])
                nc.gpsimd.dma_start(vIh_sb[:, ck4:ck4 + 4], vIh[:, ck4:ck4 + 4])
                nc.gpsimd.dma_start(vIb_sb[:, ck4:ck4 + 4], vIb[:, ck4:ck4 + 4])
            for b in range(1, NBLK):
                nc.gpsimd.dma_start(qT_sb[:, b * NQB:(b + 1) * NQB],
                                    qT[:, b * NQB:(b + 1) * NQB])
            wT_sb = const_pool.tile([HD, 2, D], fp16)
            nc.sync.dma_start(wT_sb[:], wT[:])
            zeros_sb = const_pool.tile([128, NQB], mybir.dt.bfloat16)
            nc.vector.memset(zeros_sb[:], 0.0)

            # ---- PSUM ----
            # score ring: 3 slots x [128, 1024] fp32 (2 banks each)
            ring = [ps_s.tile([128, 1024], f32, tag=f"ring{i}", name=f"ring{i}")
                    for i in range(3)]
            # accumulators / proj outputs: 2 banks, time-shared.
            acc = [ps_a.tile([128, NQB], f32, tag=f"acc{i}", name=f"acc{i}")
                   for i in range(2)]

            # HAM warm: dense burst of zero matmuls (wiped by start=True PV)
            for f in range(20):
                nc.tensor.matmul(acc[f % 2][0:HD + 1, :],
                                 lhsT=zeros_sb[:, 0:HD + 1], rhs=zeros_sb[:],
                                 start=False, stop=False)

            # ---- helpers ----
            def emit_scores(j):
                """scores matmul pair for slot j = chunk (blk, ck)."""
                blk, ck = divmod(j, NCHUNK)
                for h in range(2):
                    nc.tensor.matmul(
                        ring[j % 3][:, h * 512:(h + 1) * 512],
                        lhsT=kT_sb[h * HD:(h + 1) * HD, ck * 128:(ck + 1) * 128],
                        rhs=qT_sb[h * HD:(h + 1) * HD, blk * NQB:(blk + 1) * NQB],
                        start=True, stop=True)

            pt_of_slot = {}

            def emit_exp(j):
                src = ring[j % 3][:]
                if slot_engine(j) == "S":
                    pt = pt_pool.tile([128, 1024], fp16, tag="ptS")
                    nc.scalar.activation(pt[:], src, Exp, scale=0.125)
                else:
                    pt = ptd_pool.tile([128, 1024], i16, tag="ptD")
                    nc.vector.tensor_scalar(
                        pt[:], src, SCH_A, SCH_B, op0=mult, op1=add)
                pt_of_slot[j] = pt

            def emit_pv(blk, ck):
                j = blk * NCHUNK + ck
                pt = pt_of_slot.pop(j)
                for h in range(2):
                    sl = pt[:, h * 512:(h + 1) * 512]
                    if slot_engine(j) == "S":
                        rhs, lhsT = sl, vIh_sb[:, ck, h, :]
                    else:
                        rhs, lhsT = sl.bitcast(bf16), vIb_sb[:, ck, h, :]
                    nc.tensor.matmul(
                        acc[h][0:HD + 1, :], lhsT=lhsT, rhs=rhs,
                        start=(ck == 0), stop=(ck == NCHUNK - 1))

            def emit_stage(blk):
                """stage accumulators to SBUF + denominator transpose/recip."""
                st0 = ep_pool.tile([HD + 1, NQB], fp16, tag="st0")
                nc.vector.tensor_copy(st0[:], acc[0][0:HD + 1, :])
                dtmp = dram_pool.tile([2, NQB], fp16, tag="dtmp")
                nc.sync.dma_start(dtmp[0:1, :], st0[HD:HD + 1, :])
                st1 = ep_pool.tile([HD + 1, NQB], fp16, tag="st1")
                nc.vector.tensor_copy(st1[:], acc[1][0:HD + 1, :])
                nc.sync.dma_start(dtmp[1:2, :], st1[HD:HD + 1, :])
                dT = ep_pool.tile([128, 4, 2], fp16, tag="dT")
                for h in range(2):
                    nc.sync.dma_start(
                        dT[:, :, h], dtmp[h].rearrange("(c p) -> p c", p=128))
                denT = ep_pool.tile([128, 4, 2], f32, tag="denT")
                nc.vector.reciprocal(denT[:], dT[:])
                return st0, st1, denT

            def make_proj(cc, st0, st1, denT, blk):
                def emit_proj():
                    n0 = blk * NQB + cc * 128
                    y0 = acc[0][:, :]
                    nc.tensor.matmul(
                        y0, lhsT=st0[0:HD, cc * 128:(cc + 1) * 128],
                        rhs=wT_sb[:, 0, :], start=True, stop=True)
                    y1 = acc[1][:, :]
                    nc.tensor.matmul(
                        y1, lhsT=st1[0:HD, cc * 128:(cc + 1) * 128],
                        rhs=wT_sb[:, 1, :], start=True, stop=True)
                    t1 = ep_pool.tile([128, D], f32, tag="t1")
                    if blk == NBLK - 1:
                        # tail: ScalarE is idle (no exps left) -- split the
                        # combine across engines to shorten the serial tail
                        nc.scalar.mul(t1[:], y1, denT[:, cc, 1:2])
                    else:
                        nc.vector.tensor_scalar_mul(t1[:], y1, denT[:, cc, 1:2])
                    y_sb = ep_pool.tile([128, D], f32, tag="y_sb")
                    nc.vector.scalar_tensor_tensor(
                        y_sb[:], y0, denT[:, cc, 0:1], t1[:], op0=mult, op1=add)
                    nc.sync.dma_start(y[n0:n0 + 128, :], y_sb[:])
                return emit_proj

            # ---- main slot loop ----
            from collections import deque
            pv_ready = deque()      # (blk, ck) with both exps emitted
            pending_projs = deque()
            done_units = 0

            def drain(j):
                """emit PV work whose exp slot index < j, plus due projs."""
                nonlocal done_units
                while pending_projs and pending_projs[0][0] <= j:
                    pending_projs.popleft()[1]()
                while pv_ready and pv_ready[0][2] < j:
                    blk, ck, _ = pv_ready.popleft()
                    if ck == 0:
                        # acc banks are about to be re-armed (start=True):
                        # every proj reading them must be emitted first
                        while pending_projs:
                            pending_projs.popleft()[1]()
                    emit_pv(blk, ck)
                    done_units += 2
                    if ck == NCHUNK - 1:
                        st0, st1, denT = emit_stage(blk)
                        for cc in range(4):
                            pending_projs.append(
                                (j + 1 + cc, make_proj(cc, st0, st1, denT, blk)))

            for g in range(0, NSLOTS, 3):
                for j in range(g, min(g + 3, NSLOTS)):
                    emit_scores(j)
                for j in range(g, min(g + 3, NSLOTS)):
                    emit_exp(j)
                    blk, ck = divmod(j, NCHUNK)
                    pv_ready.append((blk, ck, j))
                drain(g)
            drain(NSLOTS + 3)
            while pending_projs:
                pending_projs.popleft()[1]()

    nc.compile()
    return nc


def _get_compiled():
    global _compiled
    if _compiled is None:
        _compiled = _build_nc()
    return _compiled


def _prep_core_inputs(x, proj_w):
    """Host-side shard + layout per core: core c -> batch c//4, head pair c%4."""
    ins = []
    for c in range(NCORES):
        b, hp = c // 4, c % 4
        sl = slice(128 * hp, 128 * hp + 128)
        qTa = np.ascontiguousarray(x[0, b, :, sl].T).astype(np.float16)
        kTa = np.ascontiguousarray(x[1, b, :, sl].T).astype(np.float16)
        v = x[2, b, :, sl]                       # [N, 128]
        vI = np.ones((128, NCHUNK, 2, HD + 1), np.float32)
        vr = v.reshape(NCHUNK, 128, 2, HD)        # [chunk, p, head, m]
        vI[:, :, :, :HD] = vr.transpose(1, 0, 2, 3)
        wTa = np.ascontiguousarray(
            proj_w[:, sl].T.reshape(2, HD, D).transpose(1, 0, 2)).astype(np.float16)
        import ml_dtypes
        ins.append({"qT": qTa, "kT": kTa, "vIh": vI.astype(np.float16),
                    "vIb": vI.astype(ml_dtypes.bfloat16), "wT": wTa})
    return ins


def kernel(x, proj_w, proj_b):
    from concourse.bass_utils import run_bass_kernel_spmd

    x = np.asarray(x, dtype=np.float32)
    proj_w = np.asarray(proj_w, dtype=np.float32)
    proj_b = np.asarray(proj_b, dtype=np.float32)

    nc = _get_compiled()
    in_maps = _prep_core_inputs(x, proj_w)
    res = run_bass_kernel_spmd(nc, in_maps, core_ids=list(range(NCORES)))

    out = np.zeros((B, N, D), np.float32)
    for c in range(NCORES):
        out[c // 4] += res.results[c]["y"]
    out += proj_b
    return out


# revision 31
# speedup vs baseline: 1.0116x; 1.0042x over previous
"""Multi-head attention + output projection on 8 Trainium2 NeuronCores.

Problem (hardcoded): x [3, 2, 4096, 512] fp32 (q/k/v stacked), proj_w [512, 512],
proj_b [512].  reference = softmax(q k^T / sqrt(64)) v, heads=8, then
out @ proj_w.T + proj_b.

Sharding: B*H = 16 (batch, head) pairs over 8 cores -> each core gets one
batch and one adjacent head PAIR (2 heads = 128 feature dims).  The output
projection is tensor-parallel along the contraction dim: each core computes
its partial y = attn_out_pair @ W[:, pair_dims].T; the host sums the 4
partials per batch and adds the bias.

Device kernel (per core).  Exp of 33.5M scores/core is the dominant cost
(ScalarE streams 1 elem/cycle/lane + ~350 cyc per ACT), so the softmax is
split across two engines and the PE stream is organized to amortize its
row-config switch penalty:
  - scores (fp16 q/k, row-tiled head pairs) land in a 6-bank PSUM ring of
    three [128, 1024] chunk slots; emission batches 3 chunks of scores,
    then 6 PV matmuls, so the K=64 <-> K=128 array-reconfig drain (~165ns)
    is paid twice per 3 chunks instead of twice per chunk.
  - exp slots cycle through EXP_PATTERN: 'S' = exact ScalarE ACT (fp16
    out), 'D' = DVE Schraudolph bit-trick exp -- one tensor_scalar
    computing int16(A*s + B) whose bits are read back as bf16, i.e.
    2^(x/log2) by exponent-field construction (~1.8% rms, metric impact
    ~1e-2 at 1/3 of keys; the denominator uses the same approximate P, so
    softmax errors partially cancel as a reweighting).
  - PV accumulates [V | 1] fp16/bf16 (denominator rides as the 65th
    column) into two PSUM banks; the projection time-shares those banks
    between blocks (all projs of block b are emitted before PV of block
    b+1 re-arms the accumulation group).
"""

import numpy as np

C, B, N, D, H = 3, 2, 4096, 512, 8
HD = 64          # head dim
NCORES = 8
NQB = 512        # nq block width
NBLK = N // NQB  # 8 q blocks
NCHUNK = N // 128  # 32 nk chunks of 128
NSLOTS = NBLK * NCHUNK             # 256 slots; 1 slot = 1 chunk = [128, 1024]

# exp engine pattern, cycled per slot: 'S' = ScalarE exact, 'D' = DVE
# Schraudolph approximation.  'SSD' puts 1/3 of exp on the DVE.
EXP_PATTERN = "SSD"

# Schraudolph constants: exp(0.125*s) ~= bitcast_bf16(int16(SCH_A*s + SCH_B))
SCH_A = float(0.125 * (1 << 7) / np.log(2))
SCH_B = float(127.0 * (1 << 7) - 7.42)

_compiled = None


def _build_nc():
    import concourse.bacc as bacc
    import concourse.tile as tile
    from concourse import mybir

    f32 = mybir.dt.float32
    f32r = mybir.dt.float32r
    fp16 = mybir.dt.float16
    bf16 = mybir.dt.bfloat16
    i16 = mybir.dt.int16
    Exp = mybir.ActivationFunctionType.Exp
    mult = mybir.AluOpType.mult
    add = mybir.AluOpType.add

    nc = bacc.Bacc("TRN2", target_bir_lowering=False, debug=False, num_devices=1)

    qT = nc.dram_tensor("qT", [128, N], fp16, kind="ExternalInput").ap()
    kT = nc.dram_tensor("kT", [128, N], fp16, kind="ExternalInput").ap()
    vIh = nc.dram_tensor("vIh", [128, NCHUNK, 2, HD + 1], fp16, kind="ExternalInput").ap()
    vIb = nc.dram_tensor("vIb", [128, NCHUNK, 2, HD + 1], bf16, kind="ExternalInput").ap()
    wT = nc.dram_tensor("wT", [HD, 2, D], fp16, kind="ExternalInput").ap()
    y = nc.dram_tensor("y", [N, D], f32, kind="ExternalOutput").ap()

    # slot -> exp engine
    def slot_engine(j):
        return EXP_PATTERN[j % len(EXP_PATTERN)]

    with tile.TileContext(nc) as tc:
        with (
            tc.tile_pool(name="const", bufs=1) as const_pool,
            tc.tile_pool(name="pt", bufs=10) as pt_pool,
            tc.tile_pool(name="ptd", bufs=5) as ptd_pool,
            tc.tile_pool(name="ep", bufs=2) as ep_pool,
            tc.tile_pool(name="ps_s", bufs=1, space="PSUM") as ps_s,
            tc.tile_pool(name="ps_a", bufs=1, space="PSUM") as ps_a,
            tc.tile_pool(name="dram", bufs=2, space="DRAM") as dram_pool,
        ):
            # ---- resident inputs ----
            qT_sb = const_pool.tile([128, N], fp16)
            kT_sb = const_pool.tile([128, N], fp16)
            vIh_sb = const_pool.tile([128, NCHUNK, 2, HD + 1], fp16)
            vIb_sb = const_pool.tile([128, NCHUNK, 2, HD + 1], bf16)
            # trigger the exp table load while the input DMAs stream in
            warm = ep_pool.tile([128, 2], f32, tag="warm")
            nc.vector.memset(warm[:], 0.0)
            nc.scalar.activation(warm[:, 1:2], warm[:, 0:1], Exp)
            nc.sync.dma_start(kT_sb[:, 0:128], kT[:, 0:128])
            nc.sync.dma_start(qT_sb[:, 0:NQB], qT[:, 0:NQB])
            nc.sync.dma_start(kT_sb[:, 128:512], kT[:, 128:512])
            nc.gpsimd.dma_start(vIh_sb[:, 0:4], vIh[:, 0:4])
            nc.gpsimd.dma_start(vIb_sb[:, 0:4], vIb[:, 0:4])
            for ck4 in range(4, NCHUNK, 4):
                sl = slice(ck4 * 128, (ck4 + 4) * 128)
                nc.sync.dma_start(kT_sb[:, sl], kT[:, sl])
                nc.gpsimd.dma_start(vIh_sb[:, ck4:ck4 + 4], vIh[:, ck4:ck4 + 4])
                nc.gpsimd.dma_start(vIb_sb[:, ck4:ck4 + 4], vIb[:, ck4:ck4 + 4])
            for b in range(1, NBLK):
                nc.gpsimd.dma_start(qT_sb[:, b * NQB:(b + 1) * NQB],
                                    qT[:, b * NQB:(b + 1) * NQB])
            wT_sb = const_pool.tile([HD, 2, D], fp16)
            nc.sync.dma_start(wT_sb[:], wT[:])
            zeros_sb = const_pool.tile([128, NQB], mybir.dt.bfloat16)
            nc.vector.memset(zeros_sb[:], 0.0)

            # ---- PSUM ----
            # score ring: 3 slots x [128, 1024] fp32 (2 banks each)
            ring = [ps_s.tile([128, 1024], f32, tag=f"ring{i}", name=f"ring{i}")
                    for i in range(3)]
            # accumulators / proj outputs: 2 banks, time-shared.
            acc = [ps_a.tile([128, NQB], f32, tag=f"acc{i}", name=f"acc{i}")
                   for i in range(2)]

            # HAM warm: dense burst of zero matmuls (wiped by start=True PV)
            for f in range(20):
                nc.tensor.matmul(acc[f % 2][0:HD + 1, :],
                                 lhsT=zeros_sb[:, 0:HD + 1], rhs=zeros_sb[:],
                                 start=False, stop=False)

            # ---- helpers ----
            def emit_scores(j):
                """scores matmul pair for slot j = chunk (blk, ck)."""
                blk, ck = divmod(j, NCHUNK)
                for h in range(2):
                    nc.tensor.matmul(
                        ring[j % 3][:, h * 512:(h + 1) * 512],
                        lhsT=kT_sb[h * HD:(h + 1) * HD, ck * 128:(ck + 1) * 128],
                        rhs=qT_sb[h * HD:(h + 1) * HD, blk * NQB:(blk + 1) * NQB],
                        start=True, stop=True)

            pt_of_slot = {}

            def emit_exp(j):
                src = ring[j % 3][:]
                if slot_engine(j) == "S":
                    pt = pt_pool.tile([128, 1024], fp16, tag="ptS")
                    nc.scalar.activation(pt[:], src, Exp, scale=0.125)
                else:
                    pt = ptd_pool.tile([128, 1024], i16, tag="ptD")
                    nc.vector.tensor_scalar(
                        pt[:], src, SCH_A, SCH_B, op0=mult, op1=add)
                pt_of_slot[j] = pt

            def emit_pv(blk, ck):
                j = blk * NCHUNK + ck
                pt = pt_of_slot.pop(j)
                for h in range(2):
                    sl = pt[:, h * 512:(h + 1) * 512]
                    if slot_engine(j) == "S":
                        rhs, lhsT = sl, vIh_sb[:, ck, h, :]
                    else:
                        rhs, lhsT = sl.bitcast(bf16), vIb_sb[:, ck, h, :]
                    nc.tensor.matmul(
                        acc[h][0:HD + 1, :], lhsT=lhsT, rhs=rhs,
                        start=(ck == 0), stop=(ck == NCHUNK - 1))

            def emit_stage(blk):
                """stage accumulators to SBUF + denominator transpose/recip."""
                st0 = ep_pool.tile([HD + 1, NQB], fp16, tag="st0")
                nc.vector.tensor_copy(st0[:], acc[0][0:HD + 1, :])
                dtmp = dram_pool.tile([2, NQB], fp16, tag="dtmp")
                nc.sync.dma_start(dtmp[0:1, :], st0[HD:HD + 1, :])
                st1 = ep_pool.tile([HD + 1, NQB], fp16, tag="st1")
                nc.vector.tensor_copy(st1[:], acc[1][0:HD + 1, :])
                nc.sync.dma_start(dtmp[1:2, :], st1[HD:HD + 1, :])
                dT = ep_pool.tile([128, 4, 2], fp16, tag="dT")
                for h in range(2):
                    nc.sync.dma_start(
                        dT[:, :, h], dtmp[h].rearrange("(c p) -> p c", p=128))
                denT = ep_pool.tile([128, 4, 2], f32, tag="denT")
                nc.vector.reciprocal(denT[:], dT[:])
                return st0, st1, denT

            def make_proj(cc, st0, st1, denT, blk):
                def emit_proj():
                    n0 = blk * NQB + cc * 128
                    y0 = acc[0][:, :]
                    nc.tensor.matmul(
                        y0, lhsT=st0[0:HD, cc * 128:(cc + 1) * 128],
                        rhs=wT_sb[:, 0, :], start=True, stop=True)
                    y1 = acc[1][:, :]
                    nc.tensor.matmul(
                        y1, lhsT=st1[0:HD, cc * 128:(cc + 1) * 128],
                        rhs=wT_sb[:, 1, :], start=True, stop=True)
                    t1 = ep_pool.tile([128, D], f32, tag="t1")
                    nc.vector.tensor_scalar_mul(t1[:], y1, denT[:, cc, 1:2])
                    y_sb = ep_pool.tile([128, D], f32, tag="y_sb")
                    nc.vector.scalar_tensor_tensor(
                        y_sb[:], y0, denT[:, cc, 0:1], t1[:], op0=mult, op1=add)
                    nc.sync.dma_start(y[n0:n0 + 128, :], y_sb[:])
                return emit_proj

            # ---- main slot loop ----
            from collections import deque
            pv_ready = deque()      # (blk, ck) with both exps emitted
            pending_projs = deque()
            done_units = 0

            def drain(j):
                """emit PV work whose exp slot index < j, plus due projs."""
                nonlocal done_units
                while pending_projs and pending_projs[0][0] <= j:
                    pending_projs.popleft()[1]()
                while pv_ready and pv_ready[0][2] < j:
                    blk, ck, _ = pv_ready.popleft()
                    if ck == 0:
                        # acc banks are about to be re-armed (start=True):
                        # every proj reading them must be emitted first
                        while pending_projs:
                            pending_projs.popleft()[1]()
                    emit_pv(blk, ck)
                    done_units += 2
                    if ck == NCHUNK - 1:
                        st0, st1, denT = emit_stage(blk)
                        for cc in range(4):
                            pending_projs.append(
                                (j + 1 + cc, make_proj(cc, st0, st1, denT, blk)))

            for g in range(0, NSLOTS, 3):
                for j in range(g, min(g + 3, NSLOTS)):
                    emit_scores(j)
                for j in range(g, min(g + 3, NSLOTS)):
                    emit_exp(j)
                    blk, ck = divmod(j, NCHUNK)
                    pv_ready.append((blk, ck, j))
                drain(g)
            drain(NSLOTS + 3)
            while pending_projs:
                pending_projs.popleft()[1]()

    nc.compile()
    return nc


def _get_compiled():
    global _compiled
    if _compiled is None:
        _compiled = _build_nc()
    return _compiled


def _prep_core_inputs(x, proj_w):
    """Host-side shard + layout per core: core c -> batch c//4, head pair c%4."""
    ins = []
    for c in range(NCORES):
        b, hp = c // 4, c % 4
        sl = slice(128 * hp, 128 * hp + 128)
        qTa = np.ascontiguousarray(x[0, b, :, sl].T).astype(np.float16)
        kTa = np.ascontiguousarray(x[1, b, :, sl].T).astype(np.float16)
        v = x[2, b, :, sl]                       # [N, 128]
        vI = np.ones((128, NCHUNK, 2, HD + 1), np.float32)
        vr = v.reshape(NCHUNK, 128, 2, HD)        # [chunk, p, head, m]
        vI[:, :, :, :HD] = vr.transpose(1, 0, 2, 3)
        wTa = np.ascontiguousarray(
            proj_w[:, sl].T.reshape(2, HD, D).transpose(1, 0, 2)).astype(np.float16)
        import ml_dtypes
        ins.append({"qT": qTa, "kT": kTa, "vIh": vI.astype(np.float16),
                    "vIb": vI.astype(ml_dtypes.bfloat16), "wT": wTa})
    return ins


def kernel(x, proj_w, proj_b):
    from concourse.bass_utils import run_bass_kernel_spmd

    x = np.asarray(x, dtype=np.float32)
    proj_w = np.asarray(proj_w, dtype=np.float32)
    proj_b = np.asarray(proj_b, dtype=np.float32)

    nc = _get_compiled()
    in_maps = _prep_core_inputs(x, proj_w)
    res = run_bass_kernel_spmd(nc, in_maps, core_ids=list(range(NCORES)))

    out = np.zeros((B, N, D), np.float32)
    for c in range(NCORES):
        out[c // 4] += res.results[c]["y"]
    out += proj_b
    return out
